# revision 38
# baseline (speedup 1.0000x reference)
"""Batched dense attention (B=16, S=2048, E=128, fp32) on 8 TRN2 NeuronCores.

Strategy (data-parallel over batch, 2 batch elements per core):
  - Load Q, K naturally ([s,e] -> SBUF [128, 2048]), PE-transpose to
    QT/KT [e=128, s=2048] (fp32).
  - scores^T tiles [k=128, q=512] = KT_tile.T @ QT_chunk via float32r
    matmuls (full rate at N=512).
  - exp on ScalarE reading PSUM, scale=1/sqrt(E) folded in, bf16 out.
    No max subtraction (scores ~ N(0,1); exp is safely bounded in fp32).
  - P@V via bf16 matmuls, lhsT = exp(scores^T) subtile [k=128, q=128],
    rhs = V' = [V | ones] [k=128, 129]; column 128 accumulates the
    softmax denominator for free.  Accumulate over k in PSUM.
  - Normalize per-partition with DVE reciprocal + tensor_scalar_mul.
"""

import numpy as np
from contextlib import ExitStack

import concourse.bass as bass
import concourse.tile as tile
from concourse import bacc, bass_utils, mybir
from concourse.masks import make_identity

B, S, E = 16, 2048, 128
N_CORES = 8
B_LOC = B // N_CORES          # batch elems per core
P = 128                       # partitions
NT = S // P                   # 16 s-tiles per batch elem
QCHUNK = 512
NQC = S // QCHUNK             # 4 q-chunks
SCALE = float(E) ** -0.5

f32 = mybir.dt.float32
f32r = mybir.dt.float32r
bf16 = mybir.dt.bfloat16
AF = mybir.ActivationFunctionType


def emit_attention(ctx: ExitStack, tc: tile.TileContext, out_ap, q_ap, k_ap, v_ap):
    nc = tc.nc

    const_pool = ctx.enter_context(tc.tile_pool(name="const", bufs=1))
    ident = const_pool.tile([P, P], f32)
    make_identity(nc, ident)
    # dtype-matched identities for transpose-mode matmuls; DVE copies count
    # as "rounding" producers for the fp32r consumer check in the verifier
    ident_r = const_pool.tile([P, P], f32r)
    nc.vector.tensor_copy(ident_r[:], ident[:])
    ident_h = const_pool.tile([P, P], bf16)
    nc.vector.tensor_copy(ident_h[:], ident[:])

    stage_pool = ctx.enter_context(tc.tile_pool(name="stage", bufs=2))
    qt_pool = ctx.enter_context(tc.tile_pool(name="qt", bufs=2))
    kt_pool = ctx.enter_context(tc.tile_pool(name="kt", bufs=2))
    vv_pool = ctx.enter_context(tc.tile_pool(name="vv", bufs=2))
    ex_pool = ctx.enter_context(tc.tile_pool(name="ex", bufs=5))
    osb_pool = ctx.enter_context(tc.tile_pool(name="osb", bufs=2))
    rcp_pool = ctx.enter_context(tc.tile_pool(name="rcp", bufs=8))
    # PSUM: scores 2x[128,1024] (4 banks) + 4 shared one-bank slots that hold
    # both the paired-accumulator tiles and the transpose staging tiles
    spsum_pool = ctx.enter_context(tc.tile_pool(name="spsum", bufs=2, space="PSUM"))
    opsum_pool = ctx.enter_context(tc.tile_pool(name="opsum", bufs=4, space="PSUM"))

    # Software pipeline: the P@V matmuls (and chunk epilogue) for iteration
    # (qc, kp) are emitted two kp steps later, so the PE never waits on the
    # exp that feeds them and always has scores matmuls in front.
    EP = E + 1  # 129

    def emit_mm2(c):
        ex, vv, accs, kp = c["ex"], c["vv"], c["accs"], c["kp"]
        for j in range(2):
            ktile = kp * 2 + j
            for qs in range(4):
                # two q-subtiles share one PSUM bank (one accumulation group)
                acc = accs[qs // 2][:, (qs % 2) * EP : (qs % 2) * EP + EP]
                nc.tensor.matmul(
                    acc,
                    ex[:, j * QCHUNK + qs * P : j * QCHUNK + (qs + 1) * P],
                    vv[:, ktile, :],
                    start=(ktile == 0 and qs % 2 == 0),
                    stop=(ktile == NT - 1 and qs % 2 == 1),
                )
        if c["last"]:
            # chunk epilogue: normalize + store this q-chunk
            accs, osb, qc, out_dr = c["accs"], c["osb"], c["qc"], c["out_dr"]
            for qs in range(4):
                rcp = rcp_pool.tile([P, 1], f32, name="rcp")
                nc.vector.reciprocal(
                    rcp[:], accs[qs // 2][:, (qs % 2) * EP + E : (qs % 2) * EP + E + 1]
                )
                nc.vector.tensor_scalar_mul(
                    osb[:, qc * 4 + qs, :],
                    accs[qs // 2][:, (qs % 2) * EP : (qs % 2) * EP + E],
                    rcp[:],
                )
            for h in range(2):
                sl = slice(qc * 4 + 2 * h, qc * 4 + 2 * h + 2)
                nc.sync.dma_start(out_dr[:, sl, :], osb[:, sl, :])

    # input-DMA / transpose chunks (s-tile ranges; small ones first so the
    # first scores matmuls can start as early as possible)
    CHUNKS = [(0, 2), (2, 4), (4, 8), (8, 12), (12, 16)]

    def make_prep(b):
        """Deferred-emission prep for batch elem b: input DMAs + transposes."""
        st = {}

        def dma_all():
            # K loaded as bf16 (SWDGE cast): bf16 weights get fast-weight-load
            # in the scores matmuls, and bf16 transposes run 1 cyc/row.
            q_nat = stage_pool.tile([P, NT, E], f32r, tag="stage", name=f"q_nat{b}")
            k_nat = stage_pool.tile([P, NT, E], bf16, tag="kstage", name=f"k_nat{b}")
            qdr = q_ap[b].rearrange("(n p) e -> p n e", p=P)
            kdr = k_ap[b].rearrange("(n p) e -> p n e", p=P)
            # first three K chunks via HWDGE f32 + DVE cast (much lower
            # latency than the SWDGE cast path) so the pipeline starts early
            kstg = stage_pool.tile([P, 8, E], f32, tag="kstg32", name=f"kstg{b}")
            for lo, hi in CHUNKS[:3]:
                nc.sync.dma_start(kstg[:, lo:hi, :], kdr[:, lo:hi, :])
                nc.scalar.dma_start(q_nat[:, lo:hi, :], qdr[:, lo:hi, :])
                nc.vector.tensor_copy(k_nat[:, lo:hi, :], kstg[:, lo:hi, :])
            # SWDGE queue: V first (the first P@V matmuls need it a couple of
            # pipeline steps in), interleaved with the remaining K chunks
            vv = vv_pool.tile([P, NT, E + 1], bf16, name=f"vv{b}")
            vdr = v_ap[b].rearrange("(n p) e -> p n e", p=P)
            nc.gpsimd.dma_start(vv[:, 0:4, 0:E], vdr[:, 0:4, :])
            vparts = [(4, 8), (8, NT)]
            for (lo, hi), (vlo, vhi) in zip(CHUNKS[3:], vparts):
                sl = slice(lo, hi)
                nc.gpsimd.dma_start(k_nat[:, sl, :], kdr[:, sl, :])
                nc.gpsimd.dma_start(vv[:, vlo:vhi, 0:E], vdr[:, vlo:vhi, :])
                nc.scalar.dma_start(q_nat[:, sl, :], qdr[:, sl, :])
            nc.vector.memset(vv[:, :, E : E + 1], 1.0)
            st["q_nat"], st["k_nat"], st["vv"] = q_nat, k_nat, vv
            # both scores operands bf16 (walrus rejects f32r x bf16 mixing);
            # the PSUM->SBUF copy casts the f32r-transposed Q to bf16
            st["qt"] = qt_pool.tile([P, S], bf16, name=f"qt{b}")
            st["kt"] = kt_pool.tile([P, S], bf16, name=f"kt{b}")

        def T(which, c):
            def go():
                if which == "q":
                    src, dst, idn, sdt = st["q_nat"], st["qt"], ident_r, f32r
                else:
                    src, dst, idn, sdt = st["k_nat"], st["kt"], ident_h, bf16
                lo, hi = CHUNKS[c]
                tp = opsum_pool.tile([P, QCHUNK], f32, tag="acc", name="tp")
                tpv = tp[:].bitcast(sdt)
                for i in range(hi - lo):
                    nc.tensor.transpose(
                        tpv[:, i * P : (i + 1) * P], src[:, lo + i, :], idn[:]
                    )
                nc.vector.tensor_copy(
                    dst[:, lo * P : hi * P], tpv[:, 0 : (hi - lo) * P]
                )
            return go

        st["dma_all"], st["T"] = dma_all, T
        return st

    from collections import deque

    weave = deque()
    carries = deque()
    preps = [make_prep(b) for b in range(B_LOC)]

    preps[0]["dma_all"]()
    preps[0]["T"]("k", 0)()
    preps[0]["T"]("q", 0)()
    preps[0]["T"]("q", 1)()
    for c in (1, 2, 3, 4):
        weave.append(preps[0]["T"]("k", c))
    for c in (2, 3, 4):
        weave.append(preps[0]["T"]("q", c))

    for b in range(B_LOC):
        st = preps[b]
        vv, qt, kt = st["vv"], st["qt"], st["kt"]
        osb = osb_pool.tile([P, NT, E], f32, name=f"osb{b}")
        out_dr = out_ap[b].rearrange("(n p) e -> p n e", p=P)

        for qc in range(NQC):
            accs = [
                opsum_pool.tile([P, 2 * EP], f32, tag="acc", name=f"acc{qs}")
                for qs in range(2)
            ]
            for kp in range(NT // 2):
                sc = spsum_pool.tile([P, 2 * QCHUNK], f32, tag="spsum", name="sc")
                for j in range(2):
                    ktile = kp * 2 + j
                    nc.tensor.matmul(
                        sc[:, j * QCHUNK : (j + 1) * QCHUNK],
                        kt[:, ktile * P : (ktile + 1) * P],
                        qt[:, qc * QCHUNK : (qc + 1) * QCHUNK],
                        start=True,
                        stop=True,
                    )
                ex = ex_pool.tile([P, 2 * QCHUNK], bf16, name="ex")
                nc.scalar.activation(ex[:], sc[:], AF.Exp, scale=SCALE)
                if len(carries) == 2:
                    emit_mm2(carries.popleft())
                carries.append(
                    dict(
                        ex=ex,
                        vv=vv,
                        accs=accs,
                        kp=kp,
                        last=(kp == NT // 2 - 1),
                        osb=osb,
                        qc=qc,
                        out_dr=out_dr,
                    )
                )
                if weave:
                    weave.popleft()()
                if b + 1 < B_LOC and qc == 1 and kp == 0:
                    preps[b + 1]["dma_all"]()
                if b + 1 < B_LOC and qc == 2 and kp == 0:
                    for wc, cc in (
                        ("k", 0), ("q", 0), ("k", 1), ("q", 1), ("k", 2),
                        ("k", 3), ("k", 4), ("q", 2), ("q", 3), ("q", 4),
                    ):
                        weave.append(preps[b + 1]["T"](wc, cc))
        # anything not yet woven must be emitted before the next batch starts
        while weave:
            weave.popleft()()
    while carries:
        emit_mm2(carries.popleft())


_CACHE: dict = {}


def build():
    if "nc" in _CACHE:
        return _CACHE["nc"]
    nc = bacc.Bacc(
        "TRN2",
        target_bir_lowering=False,
        debug=False,
        enable_asserts=False,
        num_devices=N_CORES,
    )
    q = nc.dram_tensor("q", [B_LOC, S, E], f32r, kind="ExternalInput").ap()
    k = nc.dram_tensor("k", [B_LOC, S, E], f32, kind="ExternalInput").ap()
    v = nc.dram_tensor("v", [B_LOC, S, E], f32, kind="ExternalInput").ap()
    o = nc.dram_tensor("out", [B_LOC, S, E], f32, kind="ExternalOutput").ap()
    with tile.TileContext(nc) as tc, ExitStack() as ctx:
        emit_attention(ctx, tc, o, q, k, v)
    nc.compile()
    _CACHE["nc"] = nc
    return nc


def run(query, key, value, trace=False, trace_kwargs=None):
    nc = build()
    query = np.ascontiguousarray(query, dtype=np.float32)
    key = np.ascontiguousarray(key, dtype=np.float32)
    value = np.ascontiguousarray(value, dtype=np.float32)
    in_maps = [
        {
            "q": query[c * B_LOC : (c + 1) * B_LOC],
            "k": key[c * B_LOC : (c + 1) * B_LOC],
            "v": value[c * B_LOC : (c + 1) * B_LOC],
        }
        for c in range(N_CORES)
    ]
    res = bass_utils.run_bass_kernel_spmd(
        nc,
        in_maps,
        core_ids=list(range(N_CORES)),
        trace=trace,
        **(trace_kwargs or {}),
    )
    out = np.concatenate([res.results[c]["out"] for c in range(N_CORES)], axis=0)
    return out, res


def kernel(query, key, value):
    out, _ = run(query, key, value, trace=False)
    return out


# revision 39
# speedup vs baseline: 1.1734x; 1.1734x over previous
"""Batched dense attention (B=16, S=2048, E=128, fp32) on 8 TRN2 NeuronCores.

Strategy (data-parallel over batch, 2 batch elements per core):
  - Load Q, K naturally ([s,e] -> SBUF [128, 2048]), PE-transpose to
    QT/KT [e=128, s=2048] (fp32).
  - scores^T tiles [k=128, q=512] = KT_tile.T @ QT_chunk via float32r
    matmuls (full rate at N=512).
  - exp on ScalarE reading PSUM, scale=1/sqrt(E) folded in, bf16 out.
    No max subtraction (scores ~ N(0,1); exp is safely bounded in fp32).
  - P@V via bf16 matmuls, lhsT = exp(scores^T) subtile [k=128, q=128],
    rhs = V' = [V | ones] [k=128, 129]; column 128 accumulates the
    softmax denominator for free.  Accumulate over k in PSUM.
  - Normalize per-partition with DVE reciprocal + tensor_scalar_mul.
"""

import numpy as np
from contextlib import ExitStack

import concourse.bass as bass
import concourse.tile as tile
from concourse import bacc, bass_utils, mybir
from concourse.masks import make_identity

B, S, E = 16, 2048, 128
N_CORES = 8
B_LOC = B // N_CORES          # batch elems per core
P = 128                       # partitions
NT = S // P                   # 16 s-tiles per batch elem
QCHUNK = 512
NQC = S // QCHUNK             # 4 q-chunks
SCALE = float(E) ** -0.5

f32 = mybir.dt.float32
f32r = mybir.dt.float32r
bf16 = mybir.dt.bfloat16
AF = mybir.ActivationFunctionType


def emit_attention(ctx: ExitStack, tc: tile.TileContext, out_ap, q_ap, k_ap, v_ap):
    nc = tc.nc

    const_pool = ctx.enter_context(tc.tile_pool(name="const", bufs=1))
    ident = const_pool.tile([P, P], f32)
    make_identity(nc, ident)
    # dtype-matched identities for transpose-mode matmuls; DVE copies count
    # as "rounding" producers for the fp32r consumer check in the verifier
    ident_r = const_pool.tile([P, P], f32r)
    nc.vector.tensor_copy(ident_r[:], ident[:])
    ident_h = const_pool.tile([P, P], bf16)
    nc.vector.tensor_copy(ident_h[:], ident[:])

    stage_pool = ctx.enter_context(tc.tile_pool(name="stage", bufs=2))
    qt_pool = ctx.enter_context(tc.tile_pool(name="qt", bufs=2))
    kt_pool = ctx.enter_context(tc.tile_pool(name="kt", bufs=2))
    vv_pool = ctx.enter_context(tc.tile_pool(name="vv", bufs=2))
    ex_pool = ctx.enter_context(tc.tile_pool(name="ex", bufs=5))
    osb_pool = ctx.enter_context(tc.tile_pool(name="osb", bufs=2))
    rcp_pool = ctx.enter_context(tc.tile_pool(name="rcp", bufs=8))
    # PSUM: scores 2x[128,1024] (4 banks) + 4 shared one-bank slots that hold
    # both the paired-accumulator tiles and the transpose staging tiles
    spsum_pool = ctx.enter_context(tc.tile_pool(name="spsum", bufs=2, space="PSUM"))
    opsum_pool = ctx.enter_context(tc.tile_pool(name="opsum", bufs=4, space="PSUM"))

    # Software pipeline: the P@V matmuls (and chunk epilogue) for iteration
    # (qc, kp) are emitted two kp steps later, so the PE never waits on the
    # exp that feeds them and always has scores matmuls in front.
    EP = E + 1  # 129

    def emit_mm2(c):
        ex, vv, accs, kp = c["ex"], c["vv"], c["accs"], c["kp"]
        for j in range(2):
            ktile = kp * 2 + j
            for qs in range(4):
                # two q-subtiles share one PSUM bank (one accumulation group)
                acc = accs[qs // 2][:, (qs % 2) * EP : (qs % 2) * EP + EP]
                nc.tensor.matmul(
                    acc,
                    ex[:, j * QCHUNK + qs * P : j * QCHUNK + (qs + 1) * P],
                    vv[:, ktile, :],
                    start=(ktile == 0 and qs % 2 == 0),
                    stop=(ktile == NT - 1 and qs % 2 == 1),
                )
        if c["last"]:
            # chunk epilogue: normalize + store this q-chunk
            accs, osb, qc, out_dr = c["accs"], c["osb"], c["qc"], c["out_dr"]
            for qs in range(4):
                rcp = rcp_pool.tile([P, 1], f32, name="rcp")
                nc.vector.reciprocal(
                    rcp[:], accs[qs // 2][:, (qs % 2) * EP + E : (qs % 2) * EP + E + 1]
                )
                nc.vector.tensor_scalar_mul(
                    osb[:, qc * 4 + qs, :],
                    accs[qs // 2][:, (qs % 2) * EP : (qs % 2) * EP + E],
                    rcp[:],
                )
            for h in range(2):
                sl = slice(qc * 4 + 2 * h, qc * 4 + 2 * h + 2)
                nc.sync.dma_start(out_dr[:, sl, :], osb[:, sl, :])

    # input-DMA / transpose chunks (s-tile ranges; small ones first so the
    # first scores matmuls can start as early as possible)
    CHUNKS = [(0, 2), (2, 4), (4, 8), (8, 12), (12, 16)]

    def make_prep(b):
        """Deferred-emission prep for batch elem b: input DMAs + transposes."""
        st = {}

        def dma_all():
            # K loaded as bf16 (SWDGE cast): bf16 weights get fast-weight-load
            # in the scores matmuls, and bf16 transposes run 1 cyc/row.
            q_nat = stage_pool.tile([P, NT, E], f32r, tag="stage", name=f"q_nat{b}")
            k_nat = stage_pool.tile([P, NT, E], bf16, tag="kstage", name=f"k_nat{b}")
            qdr = q_ap[b].rearrange("(n p) e -> p n e", p=P)
            kdr = k_ap[b].rearrange("(n p) e -> p n e", p=P)
            # first two K chunks via HWDGE f32 + DVE cast (much lower latency
            # than the SWDGE cast path) so the pipeline starts early
            kstg = stage_pool.tile([P, 4, E], f32, tag="kstg32", name=f"kstg{b}")
            for lo, hi in CHUNKS[:2]:
                nc.sync.dma_start(kstg[:, lo:hi, :], kdr[:, lo:hi, :])
                nc.scalar.dma_start(q_nat[:, lo:hi, :], qdr[:, lo:hi, :])
                nc.vector.tensor_copy(k_nat[:, lo:hi, :], kstg[:, lo:hi, :])
            # V goes first on the SWDGE queue (the first P@V matmuls need it
            # just a couple of pipeline steps in), split so kt 0-3 land early
            vv = vv_pool.tile([P, NT, E + 1], bf16, name=f"vv{b}")
            vdr = v_ap[b].rearrange("(n p) e -> p n e", p=P)
            vparts = [(0, 4), (4, 8), (8, NT)]
            for (lo, hi), (vlo, vhi) in zip(CHUNKS[2:], vparts):
                sl = slice(lo, hi)
                nc.gpsimd.dma_start(k_nat[:, sl, :], kdr[:, sl, :])
                nc.gpsimd.dma_start(vv[:, vlo:vhi, 0:E], vdr[:, vlo:vhi, :])
                nc.scalar.dma_start(q_nat[:, sl, :], qdr[:, sl, :])
            nc.vector.memset(vv[:, :, E : E + 1], 1.0)
            st["q_nat"], st["k_nat"], st["vv"] = q_nat, k_nat, vv
            # both scores operands bf16 (walrus rejects f32r x bf16 mixing);
            # the PSUM->SBUF copy casts the f32r-transposed Q to bf16
            st["qt"] = qt_pool.tile([P, S], bf16, name=f"qt{b}")
            st["kt"] = kt_pool.tile([P, S], bf16, name=f"kt{b}")

        def T(which, c):
            def go():
                if which == "q":
                    src, dst, idn, sdt = st["q_nat"], st["qt"], ident_r, f32r
                else:
                    src, dst, idn, sdt = st["k_nat"], st["kt"], ident_h, bf16
                lo, hi = CHUNKS[c]
                tp = opsum_pool.tile([P, QCHUNK], f32, tag="acc", name="tp")
                tpv = tp[:].bitcast(sdt)
                for i in range(hi - lo):
                    nc.tensor.transpose(
                        tpv[:, i * P : (i + 1) * P], src[:, lo + i, :], idn[:]
                    )
                nc.vector.tensor_copy(
                    dst[:, lo * P : hi * P], tpv[:, 0 : (hi - lo) * P]
                )
            return go

        st["dma_all"], st["T"] = dma_all, T
        return st

    from collections import deque

    weave = deque()
    carries = deque()
    preps = [make_prep(b) for b in range(B_LOC)]

    preps[0]["dma_all"]()
    preps[0]["T"]("k", 0)()
    preps[0]["T"]("q", 0)()
    preps[0]["T"]("q", 1)()
    for c in (1, 2, 3, 4):
        weave.append(preps[0]["T"]("k", c))
    for c in (2, 3, 4):
        weave.append(preps[0]["T"]("q", c))

    for b in range(B_LOC):
        st = preps[b]
        vv, qt, kt = st["vv"], st["qt"], st["kt"]
        osb = osb_pool.tile([P, NT, E], f32, name=f"osb{b}")
        out_dr = out_ap[b].rearrange("(n p) e -> p n e", p=P)

        for qc in range(NQC):
            accs = [
                opsum_pool.tile([P, 2 * EP], f32, tag="acc", name=f"acc{qs}")
                for qs in range(2)
            ]
            for kp in range(NT // 2):
                sc = spsum_pool.tile([P, 2 * QCHUNK], f32, tag="spsum", name="sc")
                for j in range(2):
                    ktile = kp * 2 + j
                    nc.tensor.matmul(
                        sc[:, j * QCHUNK : (j + 1) * QCHUNK],
                        kt[:, ktile * P : (ktile + 1) * P],
                        qt[:, qc * QCHUNK : (qc + 1) * QCHUNK],
                        start=True,
                        stop=True,
                    )
                ex = ex_pool.tile([P, 2 * QCHUNK], bf16, name="ex")
                nc.scalar.activation(ex[:], sc[:], AF.Exp, scale=SCALE)
                if len(carries) == 2:
                    emit_mm2(carries.popleft())
                carries.append(
                    dict(
                        ex=ex,
                        vv=vv,
                        accs=accs,
                        kp=kp,
                        last=(kp == NT // 2 - 1),
                        osb=osb,
                        qc=qc,
                        out_dr=out_dr,
                    )
                )
                if weave:
                    weave.popleft()()
                if b + 1 < B_LOC and qc == 1 and kp == 0:
                    preps[b + 1]["dma_all"]()
                if b + 1 < B_LOC and qc == 2 and kp == 0:
                    for wc, cc in (
                        ("k", 0), ("q", 0), ("k", 1), ("q", 1), ("k", 2),
                        ("k", 3), ("k", 4), ("q", 2), ("q", 3), ("q", 4),
                    ):
                        weave.append(preps[b + 1]["T"](wc, cc))
        # anything not yet woven must be emitted before the next batch starts
        while weave:
            weave.popleft()()
    while carries:
        emit_mm2(carries.popleft())


_CACHE: dict = {}


def build():
    if "nc" in _CACHE:
        return _CACHE["nc"]
    nc = bacc.Bacc(
        "TRN2",
        target_bir_lowering=False,
        debug=False,
        enable_asserts=False,
        num_devices=N_CORES,
    )
    q = nc.dram_tensor("q", [B_LOC, S, E], f32r, kind="ExternalInput").ap()
    k = nc.dram_tensor("k", [B_LOC, S, E], f32, kind="ExternalInput").ap()
    v = nc.dram_tensor("v", [B_LOC, S, E], f32, kind="ExternalInput").ap()
    o = nc.dram_tensor("out", [B_LOC, S, E], f32, kind="ExternalOutput").ap()
    with tile.TileContext(nc) as tc, ExitStack() as ctx:
        emit_attention(ctx, tc, o, q, k, v)
    nc.compile()
    _CACHE["nc"] = nc
    return nc


def run(query, key, value, trace=False, trace_kwargs=None):
    nc = build()
    query = np.ascontiguousarray(query, dtype=np.float32)
    key = np.ascontiguousarray(key, dtype=np.float32)
    value = np.ascontiguousarray(value, dtype=np.float32)
    in_maps = [
        {
            "q": query[c * B_LOC : (c + 1) * B_LOC],
            "k": key[c * B_LOC : (c + 1) * B_LOC],
            "v": value[c * B_LOC : (c + 1) * B_LOC],
        }
        for c in range(N_CORES)
    ]
    res = bass_utils.run_bass_kernel_spmd(
        nc,
        in_maps,
        core_ids=list(range(N_CORES)),
        trace=trace,
        **(trace_kwargs or {}),
    )
    out = np.concatenate([res.results[c]["out"] for c in range(N_CORES)], axis=0)
    return out, res


def kernel(query, key, value):
    out, _ = run(query, key, value, trace=False)
    return out
